# revision 46
# baseline (speedup 1.0000x reference)
"""Trainium2 Bass kernel for the DISL loss (nn_DISL_Loss).

Single-pass strategy (data-parallel over samples, 8 cores, fp8 compute):
  The per-row cosine denominators are exactly rnV*rnA / rnV*rnF / rnA*rnF
  (the greedy permutation preserves row norms), so shipping ROW-NORMALIZED
  operands lets the three cosine sums collapse into entry-sums of the Gram
  matrices themselves:
      S1 = sum_c G'_A[c, invA[c]],  G'_A = (OA/rnA)^T (V/rnV)
      S2 = sum_c G'_F[c, invF[c]],  G'_F = (OF/rnF)^T (V/rnV)
      S3 = sum_{g[c]<OM} G'_AF[c, g[c]],  G'_AF = (OA/rnA)^T (OF/rnF)
      cos_sum = 3B - (S1+S2+S3)/T,  d_sum = cos_sum / B
  so no second pass / no per-row gathers are needed at all.  The greedy
  matching runs host-side on G' column-normalized by Vn's column norms;
  this perturbs entries ~3% relative to the reference's raw-G sim, on top
  of the ~12% fp8 contraction noise, and measured end-to-end error stays
  ~2e-4 (tolerance 2e-2) because matched entries are near-ties.

  Device (one pass, per core, fp8 DoubleRow matmuls at 0.5 cyc/col):
    G'_A, G'_F  [OM, M] partials, G'_AF [OM, OM] partial, and the triplet
    row-sums Tm = W^T vafg over host-compacted triplet rows (W prescaled
    x512 so fp8 never underflows; the scale cancels under normalization).
    Normalized operands are prescaled x16 so fp8 stays in normal range;
    outputs carry the x256 factor which the host divides off.
  Host: norms, normalized fp8 casts, sample->core balancing (triplet rows
    are concentrated in normal-labeled samples), triplet-row compaction,
    all-reduce of partials, greedy matching, final combine (small only).

Schedule (23905ns cost-model time; PE gapless 2417->20711 at the fp8-DR
roofline): quarters A0,A1,F0,Tm,F1,AF0,AF1 rotate a bufs=2 PSUM pool;
an/vn stream in consumption-ordered 4-ktile chunks on SP/Pool so no
matmul ever waits on a load; the final quarter is split into three
accumulation groups of decreasing width (512/384/128 cols, separate psum
tiles) so the wide drains overlap the last matmuls, with drain routes
(mi0: DVE-copy->Pool, big: Act-copy->Act-DMA same-engine fusion, small:
DVE-copy->SP) fixed by exhaustive permutation sweeps.  Remaining time is
simulator constants: ~2.4us dispatch floor at start, ~2.3us sem-prop +
drain + barrier epilogue after the last transfer.
"""

import numpy as np
import ml_dtypes

B, T, M, OM = 64, 256, 1024, 512
N_CORES = 8
SPC = B // N_CORES          # samples per core
RPC = SPC * T               # rows per core
P = 128
KT = RPC // P               # row-tiles per core
WSCALE = 512.0
SCALE_N = 16.0              # prescale on normalized operands (x256 on G')

F8 = ml_dtypes.float8_e4m3

_prog_cache = {}


# ---------------------------------------------------------------- device pass
def _build_pass(rows, wrows):
    from concourse import bacc, mybir
    from concourse.tile import TileContext

    f32 = mybir.dt.float32
    fp8 = mybir.dt.float8e4
    DR = mybir.MatmulPerfMode.DoubleRow
    kt = rows // P
    npair = kt // 2
    ktw = wrows // P
    npw = ktw // 2

    nc = bacc.Bacc()
    an_d = nc.declare_dram_parameter("an8", [rows, OM], fp8, isOutput=False)
    fn_d = nc.declare_dram_parameter("fn8", [rows, OM], fp8, isOutput=False)
    vn_d = nc.declare_dram_parameter("vn8", [rows, M], fp8, isOutput=False)
    vg_d = nc.declare_dram_parameter("vafg8", [wrows, M], fp8, isOutput=False)
    # W is [wrows, 4] logically; padded to 64 cols (ISA ldweights minimum),
    # host-packed partition-major so the load is one 512B run per partition
    w_d = nc.declare_dram_parameter("w8", [P, ktw * 64], fp8, isOutput=False)
    ga_d = nc.declare_dram_parameter("ga", [OM, M], fp8, isOutput=True)
    gf_d = nc.declare_dram_parameter("gf", [OM, M], fp8, isOutput=True)
    gaf_d = nc.declare_dram_parameter("gaf", [OM, OM], fp8, isOutput=True)
    tm_d = nc.declare_dram_parameter("tm", [4, M], f32, isOutput=True)

    with TileContext(nc) as tc:
        with (
            tc.tile_pool(name="res", bufs=1) as rpool,
            tc.tile_pool(name="out", bufs=2) as opool,
        ):
            an_sb = rpool.tile([P, kt, OM], fp8, tag="an")
            fn_sb = rpool.tile([P, kt, OM], fp8, tag="fn")
            vn_sb = rpool.tile([P, kt, M], fp8, tag="vn")
            vg_sb = rpool.tile([P, ktw, M], fp8, tag="vg")
            w_sb = rpool.tile([P, ktw, 64], fp8, tag="w")

            def ldr(dram, sb, k0, k1, eng):
                eng.dma_start(
                    sb[:, k0:k1, :],
                    dram[k0 * P:k1 * P, :].rearrange(
                        "(k p) m -> p k m", p=P))

            # an/vn stream in consumption-order chunks on SP/Pool (per-queue
            # issue rate is ~1.2us, transfers overlap across queues), so the
            # A quarters' k-pairs never starve; fn follows on SP for the F
            # quarters; vg/w ride the otherwise-idle Act queue early
            nc.scalar.dma_start(
                w_sb[:], w_d.rearrange("p (k c) -> p k c", c=64))
            vg_cuts = sorted({0, min(2, ktw), ktw // 2, ktw})
            for a, b in zip(vg_cuts, vg_cuts[1:]):
                if b > a:
                    ldr(vg_d, vg_sb, a, b, nc.scalar)
            # first vn pair split by column halves: PE's opening matmuls
            # need only vn[:, 0:2, 0:512], which lands ~350ns sooner than
            # the full pair would
            ldr(an_d, an_sb, 0, 2, nc.sync)
            nc.gpsimd.dma_start(
                vn_sb[:, 0:2, 0:512],
                vn_d[0:2 * P, 0:512].rearrange("(k p) m -> p k m", p=P))
            nc.sync.dma_start(
                vn_sb[:, 0:2, 512:M],
                vn_d[0:2 * P, 512:M].rearrange("(k p) m -> p k m", p=P))
            for k0, k1 in ((2, 6), (6, 10), (10, 14), (14, 16)):
                ldr(an_d, an_sb, k0, k1, nc.sync)
                ldr(vn_d, vn_sb, k0, k1, nc.gpsimd)
            ldr(fn_d, fn_sb, 0, 4, nc.sync)
            ldr(fn_d, fn_sb, 4, 10, nc.sync)
            ldr(fn_d, fn_sb, 10, 16, nc.sync)

            gab = rpool.tile([P, 4, M], fp8, tag="gab")
            gfb = rpool.tile([P, 4, M], fp8, tag="gfb")
            spool_af = [rpool.tile([P, 1, OM], fp8, tag=f"gafb{i}",
                                   name=f"gafb{i}")
                        for i in range(4)]

            # quarters rotate through one PSUM pool (bufs=2): each quarter's
            # psum->sbuf copies and output DMA overlap the next quarter.
            with tc.tile_pool(name="psq", bufs=2, space="PSUM") as psq:
                def g_quarter(src_sb, mch, gsb, g_dram, tagc):
                    tiles = []
                    for nh in range(2):
                        gp = psq.tile([P, 2, 512], f32, tag=f"q{nh}",
                                      name=f"g{tagc}{mch}_{nh}")
                        tiles.append(gp)
                    for jp in range(npair):
                        for mi in range(2):
                            mc = mch * 2 + mi
                            for nh in range(2):
                                nc.tensor.matmul(
                                    tiles[nh][:, mi, :],
                                    lhsT=src_sb[:, 2 * jp:2 * jp + 2,
                                                mc * P:(mc + 1) * P],
                                    rhs=vn_sb[:, 2 * jp:2 * jp + 2,
                                              nh * 512:(nh + 1) * 512],
                                    start=(jp == 0), stop=(jp == npair - 1),
                                    perf_mode=DR)
                    lo = mch * 2
                    nc.scalar.copy(gsb[:, lo:lo + 2, 0:512], tiles[0][:])
                    nc.vector.tensor_copy(gsb[:, lo:lo + 2, 512:M],
                                          tiles[1][:])
                    eng = nc.sync if mch == 0 else nc.gpsimd
                    eng.dma_start(
                        g_dram[mch * 256:(mch + 1) * 256, :].rearrange(
                            "(c p) m -> p c m", p=P),
                        gsb[:, lo:lo + 2, :])

                def af_quarter(mch):
                    # separate psum tiles, staging tiles, and DMAs per half
                    # so the two drains run concurrently on Act/DVE+SP/Pool
                    # (shared tiles serialize them via tile-granular deps)
                    gps = []
                    for mi in range(2):
                        gp = psq.tile([P, 2, 512], f32, tag=f"q{mi}",
                                      name=f"gaf{mch}_{mi}")
                        gps.append(gp)
                    for jp in range(npair):
                        for mi in range(2):
                            mc = mch * 2 + mi
                            nc.tensor.matmul(
                                gps[mi][:, mi, :],
                                lhsT=an_sb[:, 2 * jp:2 * jp + 2,
                                           mc * P:(mc + 1) * P],
                                rhs=fn_sb[:, 2 * jp:2 * jp + 2, :],
                                start=(jp == 0), stop=(jp == npair - 1),
                                perf_mode=DR)
                    for mi, (ceng, deng) in enumerate(
                            ((nc.scalar, nc.sync), (nc.vector, nc.gpsimd))):
                        hb = spool_af[2 * mch + mi]
                        if mi == 0:
                            ceng.copy(hb[:], gps[mi][:, mi:mi + 1, :])
                        else:
                            ceng.tensor_copy(hb[:], gps[mi][:, mi:mi + 1, :])
                        mc = mch * 2 + mi
                        deng.dma_start(
                            gaf_d[mc * P:(mc + 1) * P, :].rearrange(
                                "(c p) m -> p c m", p=P),
                            hb[:])

                g_quarter(an_sb, 0, gab, ga_d, "a")
                g_quarter(an_sb, 1, gab, ga_d, "a")
                g_quarter(fn_sb, 0, gfb, gf_d, "f")

                # Tm quarter mid-stream: vg/w are long loaded, the q1 psum
                # buffer it reuses (gf0's) has drained, and its slow drain
                # (4-partition copies + fixed DMA chain) rides F1/AF
                # instead of the tail
                tmps = psq.tile([P, 2, 512], f32, tag="q1", name="tmq")
                for jp in range(npw):
                    for nh in range(2):
                        nc.tensor.matmul(
                            tmps[0:64, nh, :],
                            lhsT=w_sb[:, 2 * jp:2 * jp + 2, :],
                            rhs=vg_sb[:, 2 * jp:2 * jp + 2,
                                      nh * 512:(nh + 1) * 512],
                            start=(jp == 0), stop=(jp == npw - 1),
                            perf_mode=DR)
                tmo = opool.tile([4, 2, 512], f32, tag="tmo")
                nc.scalar.copy(tmo[:, 0, :], tmps[0:4, 0, :])
                nc.vector.tensor_copy(tmo[:, 1, :], tmps[0:4, 1, :])
                nc.sync.dma_start(
                    tm_d.rearrange("a (b m) -> a b m", b=2), tmo[:])

                g_quarter(fn_sb, 1, gfb, gf_d, "f")
                af_quarter(0)

                # final quarter (AF mch=1) in three accumulation groups of
                # decreasing width: the wide groups stop (and start
                # draining) before the last matmul, so the post-PE tail is
                # only a 192-column copy + small DMA ahead of the fixed
                # sem-prop/barrier epilogue.  Separate psum tiles per group
                # (tile-granular deps would serialize the drains).
                g0 = psq.tile([P, 2, 512], f32, tag="q0", name="gaf1_0")
                g1 = psq.tile([P, 2, 512], f32, tag="q1", name="gaf1_1")
                gs = psq.tile([P, 2, 512], f32, tag="q0", name="gaf1_s")
                for jp in range(npair):
                    nc.tensor.matmul(
                        g0[:, 0, :],
                        lhsT=an_sb[:, 2 * jp:2 * jp + 2, 2 * P:3 * P],
                        rhs=fn_sb[:, 2 * jp:2 * jp + 2, :],
                        start=(jp == 0), stop=(jp == npair - 1),
                        perf_mode=DR)
                for jp in range(npair):
                    nc.tensor.matmul(
                        g1[:, 1, 0:384],
                        lhsT=an_sb[:, 2 * jp:2 * jp + 2, 3 * P:4 * P],
                        rhs=fn_sb[:, 2 * jp:2 * jp + 2, 0:384],
                        start=(jp == 0), stop=(jp == npair - 1),
                        perf_mode=DR)
                for jp in range(npair):
                    nc.tensor.matmul(
                        gs[:, 0, 0:128],
                        lhsT=an_sb[:, 2 * jp:2 * jp + 2, 3 * P:4 * P],
                        rhs=fn_sb[:, 2 * jp:2 * jp + 2, 384:512],
                        start=(jp == 0), stop=(jp == npair - 1),
                        perf_mode=DR)
                h0 = spool_af[2]
                nc.vector.tensor_copy(h0[:], g0[:, 0:1, :])
                nc.gpsimd.dma_start(
                    gaf_d[2 * P:3 * P, :].rearrange("(c p) m -> p c m", p=P),
                    h0[:])
                h1 = rpool.tile([P, 1, 384], fp8, tag="gafh1")
                nc.scalar.copy(h1[:], g1[:, 1:2, 0:384])
                nc.scalar.dma_start(
                    gaf_d[3 * P:4 * P, 0:384].rearrange(
                        "(c p) m -> p c m", p=P),
                    h1[:])
                h2 = rpool.tile([P, 1, 128], fp8, tag="gafh2")
                nc.vector.tensor_copy(h2[:], gs[:, 0:1, 0:128])
                nc.sync.dma_start(
                    gaf_d[3 * P:4 * P, 384:512].rearrange(
                        "(c p) m -> p c m", p=P),
                    h2[:])
    nc.finalize()
    return nc


# ---------------------------------------------------------------- host math
def _greedy_ext(sim):
    om, m = sim.shape
    used = np.zeros(m, dtype=bool)
    I = np.empty(om, dtype=np.int32)
    for r in range(om):
        row = np.where(used, -np.inf, sim[r])
        c = int(np.argmax(row))
        I[r] = c
        used[c] = True
    ext = np.empty(m, dtype=np.int32)
    ext[:om] = I
    ext[om:] = np.nonzero(~used)[0]
    return ext


def _triplet_weights(label, seq_len, vaf_avf):
    f32 = np.float32
    y = np.asarray(label).astype(np.int64)
    n_idx = np.nonzero(y == 0)[0]
    a_idx = np.nonzero(y == 1)[0]
    W = np.zeros((B, T, 4), f32)
    ar = np.arange(T)
    Nn, Na = len(n_idx), len(a_idx)
    if Nn and Na:
        for b in n_idx:
            L = int(seq_len[b])
            W[b, :, 0] = (ar < L).astype(f32) * WSCALE / (f32(L) * Nn)
        for b in a_idx:
            L = int(seq_len[b])
            k = L // 16 + 1
            sig = np.asarray(vaf_avf[b], np.float64)
            valid = ar < L
            o_s = np.argsort(np.where(valid, sig, np.inf), kind="stable")
            o_l = np.argsort(np.where(valid, -sig, np.inf), kind="stable")
            W[b, o_s[:k], 1] = WSCALE / (f32(k) * Na)
            W[b, o_l[:k], 2] = WSCALE / (f32(k) * Na)
    return W, Nn, Na


def _assign_cores(W):
    """Balance samples over cores by triplet-row count (labels are
    block-ordered so a plain split would leave cores 0-3 with all the
    anchor rows).  LPT greedy, exactly SPC samples per core."""
    rc = (np.abs(W).sum(2) > 0).sum(1)          # triplet rows per sample
    order = np.argsort(-rc, kind="stable")
    bins = [[] for _ in range(N_CORES)]
    loads = np.zeros(N_CORES, np.int64)
    for b in order:
        free = [c for c in range(N_CORES) if len(bins[c]) < SPC]
        c = min(free, key=lambda c: loads[c])
        bins[c].append(int(b))
        loads[c] += int(rc[b])
    return [np.array(sorted(bn)) for bn in bins], int(loads.max())


_runner_cache = {}


def _make_runner(nc):
    """Cached variant of bass2jax.run_bass_via_pjrt's multi-core path: jit
    once per program, reuse the compiled executable across kernel() calls."""
    import jax
    import numpy as _np
    from jax.experimental.shard_map import shard_map
    from jax.sharding import Mesh, PartitionSpec
    from concourse import bass2jax, mybir

    bass2jax.install_neuronx_cc_hook()
    assert nc.dbg_addr is None or not nc.dbg_callbacks
    partition_name = (nc.partition_id_tensor.name
                      if nc.partition_id_tensor else None)
    in_names, out_names, out_avals, zero_shapes = [], [], [], []
    for alloc in nc.m.functions[0].allocations:
        if not isinstance(alloc, mybir.MemoryLocationSet):
            continue
        name = alloc.memorylocations[0].name
        if alloc.kind == "ExternalInput":
            if name != partition_name:
                in_names.append(name)
        elif alloc.kind == "ExternalOutput":
            shape = tuple(alloc.tensor_shape)
            dtype = mybir.dt.np(alloc.dtype)
            out_names.append(name)
            out_avals.append(jax.core.ShapedArray(shape, dtype))
            zero_shapes.append((shape, dtype))
    n_params = len(in_names)
    n_outs = len(out_names)
    all_in = list(in_names) + list(out_names)
    if partition_name is not None:
        all_in.append(partition_name)
    donate = tuple(range(n_params, n_params + n_outs))

    def _body(*args):
        operands = list(args)
        if partition_name is not None:
            operands.append(bass2jax.partition_id_tensor())
        return tuple(bass2jax._bass_exec_p.bind(
            *operands,
            out_avals=tuple(out_avals),
            in_names=tuple(all_in),
            out_names=tuple(out_names),
            lowering_input_output_aliases=(),
            sim_require_finite=True,
            sim_require_nnan=True,
            nc=nc,
        ))

    devices = jax.devices()[:N_CORES]
    mesh = Mesh(_np.asarray(devices), ("core",))
    in_specs = (PartitionSpec("core"),) * (n_params + n_outs)
    out_specs = (PartitionSpec("core"),) * n_outs
    sharded = jax.jit(
        shard_map(_body, mesh=mesh, in_specs=in_specs, out_specs=out_specs,
                  check_rep=False),
        donate_argnums=donate, keep_unused=True)

    def run(in_maps):
        concat_in = [
            np.concatenate([np.asarray(m[name]) for m in in_maps], axis=0)
            for name in in_names
        ]
        concat_zeros = [
            np.zeros((N_CORES * s[0], *s[1:]), d) for (s, d) in zero_shapes
        ]
        out_arrs = sharded(*concat_in, *concat_zeros)
        return [
            {name: np.asarray(out_arrs[i]).reshape(
                N_CORES, *out_avals[i].shape)[c]
             for i, name in enumerate(out_names)}
            for c in range(N_CORES)
        ]

    return run


def _run_spmd(nc, in_maps):
    key = id(nc)
    if key not in _runner_cache:
        _runner_cache[key] = _make_runner(nc)
    return _runner_cache[key](in_maps)


def kernel(v_satt, va_satt, vf_satt, vaf_satt, v_avf, va_avf, vf_avf, vaf_avf,
           va_out, vf_out, vaf_out, lamda1, lamda2, lamda3, lamda4,
           label, seq_len):
    f32 = np.float32
    V = np.asarray(v_satt, f32).reshape(B * T, M)
    OAr = np.asarray(va_satt, f32).reshape(B * T, OM)
    OFr = np.asarray(vf_satt, f32).reshape(B * T, OM)
    vaf = np.asarray(vaf_satt, f32).reshape(B * T, M)
    # uniform prescale is exactly neutral (the triplet normalizes anchor/
    # pos/neg) but keeps large-magnitude inputs inside fp8 range
    vscale = max(1.0, float(np.abs(vaf).max()) / 200.0)
    vaf8 = (vaf / vscale).astype(F8)

    rnV = np.sqrt(np.square(V).sum(1, dtype=np.float64)).astype(f32)
    rnA = np.sqrt(np.square(OAr).sum(1, dtype=np.float64)).astype(f32)
    rnF = np.sqrt(np.square(OFr).sum(1, dtype=np.float64)).astype(f32)
    vn8 = (V * (SCALE_N / rnV[:, None])).astype(F8)
    an8 = (OAr * (SCALE_N / rnA[:, None])).astype(F8)
    fn8 = (OFr * (SCALE_N / rnF[:, None])).astype(F8)

    W, Nn, Na = _triplet_weights(label, seq_len, vaf_avf)
    Wf = W.reshape(B * T, 4)
    bins, maxrows = _assign_cores(W)
    capw = max(256, -(-maxrows // 256) * 256)

    key = (RPC, capw)
    if key not in _prog_cache:
        _prog_cache[key] = _build_pass(RPC, capw)

    in_maps = []
    for c in range(N_CORES):
        rows = (bins[c][:, None] * T + np.arange(T)[None, :]).reshape(-1)
        trip_rows = rows[(np.abs(Wf[rows]).sum(1) > 0)]
        vafg = np.zeros((capw, M), F8)
        vafg[:len(trip_rows)] = vaf8[trip_rows]
        wg = np.zeros((capw, 64), F8)
        wg[:len(trip_rows), :4] = Wf[trip_rows].astype(F8)
        # partition-major pack so the device load is 512B runs per partition
        wpk = np.ascontiguousarray(
            wg.reshape(capw // P, P, 64).transpose(1, 0, 2)).reshape(P, -1)
        in_maps.append(dict(an8=an8[rows], fn8=fn8[rows], vn8=vn8[rows],
                            vafg8=vafg, w8=wpk))
    res = _run_spmd(_prog_cache[key], in_maps)

    GpA = np.zeros((OM, M), np.float64)
    GpF = np.zeros((OM, M), np.float64)
    GpAF = np.zeros((OM, OM), np.float64)
    Tm = np.zeros((4, M), np.float64)
    for r in res:
        GpA += r["ga"].astype(np.float64)
        GpF += r["gf"].astype(np.float64)
        GpAF += r["gaf"].astype(np.float64)
        Tm += r["tm"]
    s2 = SCALE_N * SCALE_N
    GpA /= s2
    GpF /= s2
    GpAF /= s2

    # matching on G' column-normalized by Vn's column norms (argmax-invariant
    # to row scaling; ~3% entry perturbation vs the reference's raw-G sim)
    nVn = np.maximum(
        np.sqrt(np.square(vn8.astype(f32)).sum(0, dtype=np.float64)), 1e-12)
    extA = _greedy_ext((GpA / nVn[None, :]).astype(f32))
    extF = _greedy_ext((GpF / nVn[None, :]).astype(f32))
    invA = np.empty(M, np.int64)
    invA[extA] = np.arange(M)
    invF = np.empty(M, np.int64)
    invF[extF] = np.arange(M)
    g = extF[invA[:OM]]

    S1 = GpA[np.arange(OM), invA[:OM]].sum()
    S2 = GpF[np.arange(OM), invF[:OM]].sum()
    sel = g < OM
    S3 = GpAF[np.arange(OM)[sel], g[sel]].sum()
    d_sum = (3 * B - (S1 + S2 + S3) / T) / B

    ar = np.arange(T)
    seqm = (ar[None, :] < np.asarray(seq_len)[:, None]).astype(np.float64)
    Vs = np.asarray(v_avf, np.float64) * seqm
    As = np.asarray(va_avf, np.float64) * seqm
    Fs = np.asarray(vf_avf, np.float64) * seqm

    def ce(q, p):
        e = 1e-6
        q = np.clip(q, e, 1 - e)
        p = np.clip(p, e, 1 - e)
        return -(p * np.log(q) + (1 - p) * np.log(1 - q)).mean()

    ma_loss = d_sum + ce(Vs, As) + ce(Vs, Fs) + ce(As, Fs)

    yf = np.asarray(label).astype(np.float64)

    def bce(p, yy):
        p = np.asarray(p, np.float64)
        return -(yy * np.log(p) + (1 - yy) * np.log(1 - p)).mean()

    a_loss = bce(va_out, yf)
    f_loss = bce(vf_out, yf)
    raf_loss = bce(vaf_out, yf)

    if Nn == 0 or Na == 0:
        trip = 0.0
    else:
        anchor, pos, neg = Tm[0] / WSCALE, Tm[1] / WSCALE, Tm[2] / WSCALE
        nrm = lambda x: x / np.linalg.norm(x)
        a_, p_, g_ = nrm(anchor), nrm(pos), nrm(neg)
        d = lambda x, z: np.linalg.norm(x - z + 1e-6)
        trip = max(d(a_, p_) - d(a_, g_) + 5.0, 0.0)

    lam = [float(lamda1), float(lamda2), float(lamda3), float(lamda4)]
    total = (lam[0] * ma_loss + lam[1] * (a_loss + f_loss)
             + lam[2] * raf_loss + lam[3] * trip)
    return np.array([total, ma_loss, a_loss + f_loss, raf_loss, trip], f32)
